# revision 10
# baseline (speedup 1.0000x reference)
"""Trainium2 Bass kernel for nn_CLFBlock (linear -> LIF scan -> linear -> T-mean -> log_softmax).

Self-contained: hardcodes shapes T=32, B=512, D=1024, C=1000 and data-parallel
sharding of the batch dim across 8 NeuronCores.

Math notes:
  h = x @ W1.T + b1                      (fp8 on the PE, fp32 accum)
  LIF (tau=2, v_th=1, hard reset to 0):
     v' = 0.5*v + 0.5*h
     s  = (v' >= 1);  v = v' * (v' < 1)
  Scan state is the pre-reset voltage w_t, kept ping-ponged between two
  buffers so consecutive steps have no write-after-read hazard:
     w_t = select(w_{t-1} < 1, w_{t-1}, 0) * 0.5 + hh_t   (one fused DVE op)
     m_t = (w_t < 1)                                       (DVE tensor_scalar)
  Spike sum accumulates on the tensor engine: msum_psum += I @ m_t, and
  sum_t s_t = T - msum.
  y = (sum_t s_t) @ W2.T / T + b2
  out = log_softmax(y, axis=1)

Layout: the tensor engine contracts along the partition axis, so both matmul
operands need the contraction dim (d / e) on partitions. The host-side shard
step hands each core its x slice already transposed ([D, T*Bc]) and the
weights transposed once ([D, D] / [D, C]); everything loads as fp8 in its
final layout (weights are pre-scaled by 256 on the host so the uniform
(-1/32,1/32) values sit in fp8e4m3's normal range; compensated in the h-copy
and output scales).

Schedule: DMAs are issued in parallel from three rings right at engine boot
(sync: W1 halves + x tail; scalar: x head chunks; gpsimd/SWDGE: b1/b2/W2),
ordered so the first mm1 group's operands land first.  mm1 groups, the LIF
scan and the spike-sum matmuls are interleaved so every engine streams
without stalls, and a single manually placed ACT table load
(natural_log_exp_and_others) covers every activation in the kernel --
including the epilogue's Exp/Ln -- so no mid-kernel table switches occur.
The epilogue computes exp directly from mm2's PSUM with per-row accumulation
(no separate reduce pass input), ln of the row sums, and a fused
scale-subtract from PSUM, overlapping the second mm2 half with the first
half's exp and splitting the output DMA so it starts early.
"""

import numpy as np
from contextlib import ExitStack

import concourse.bass as bass
import concourse.tile as tile
from concourse import bacc, mybir
from concourse.bass_utils import run_bass_kernel_spmd

N_CORES = 8


def _lif_op():
    """Fused LIF step as a custom DVE op:
         out = select(in0 < s0, in0, 0) * s1 + in1
       i.e. w_new = reset(w_old)*0.5 + hh  in a single VectorE instruction."""
    from concourse import dve_ops
    from concourse.dve_spec import Spec, Src0, Src1, Zero, C0, C1, select, lower
    from concourse.dve_uop import DveOpSpec

    for op in dve_ops.OPS:
        if op.name == "LIF_STEP_ANT":
            return op
    spec = Spec(
        body=select(Src0 < C0, Src0, Zero) * C1 + Src1,
        reference=lambda in0, in1, s0, s1, imm2: (
            np.where(in0.astype(np.float32) < s0, in0.astype(np.float32), 0.0) * s1
            + in1.astype(np.float32)).astype(np.float32),
    )
    row = dve_ops._CUSTOM_DVE_ROW_BASE + len(dve_ops.OPS)
    shas = {}
    for ver in ("v3", "v4"):
        try:
            shas[ver] = DveOpSpec(name="LIF_STEP_ANT", opcode=row,
                                  uops=lower(spec, ver=ver), rd1_en=True).sha(ver)
        except Exception:
            pass
    op = dve_ops.DveOp("LIF_STEP_ANT", spec, subdim=False, uops_sha=shas)
    dve_ops.OPS.append(op)
    dve_ops._SUB_OPCODE_FOR_NAME[op.name] = row
    dve_ops.CUSTOM_DVE_SPECS[op.name] = spec
    return op


T, B, D, C = 32, 512, 1024, 1000
BC = B // N_CORES          # 64 rows per core
TB = T * BC                # 2048 matmul rows per core
FP32 = mybir.dt.float32
BF16 = mybir.dt.bfloat16
FP8 = mybir.dt.float8e4
W1_PRESCALE = 256.0
AF = mybir.ActivationFunctionType
OP = mybir.AluOpType
NATURAL_LOG_EXP_SET = 6    # index of natural_log_exp_and_others in act_info.json


def build_program():
    nc = bacc.Bacc("TRN2", target_bir_lowering=False, debug=False, num_devices=N_CORES)

    xt_d = nc.dram_tensor("xT", [D, TB], FP8, kind="ExternalInput").ap()
    w1t_d = nc.dram_tensor("W1T", [D, D], FP8, kind="ExternalInput").ap()
    b1_d = nc.dram_tensor("b1", [D], FP32, kind="ExternalInput").ap()
    w2t_d = nc.dram_tensor("W2T", [D, C], FP8, kind="ExternalInput").ap()
    b2_d = nc.dram_tensor("b2", [C], FP32, kind="ExternalInput").ap()
    y_d = nc.dram_tensor("y", [BC, C], FP32, kind="ExternalOutput").ap()

    with tile.TileContext(nc) as tc, ExitStack() as ctx:
        persist = ctx.enter_context(tc.tile_pool(name="persist", bufs=1))
        small = ctx.enter_context(tc.tile_pool(name="small", bufs=1))
        ps_h = ctx.enter_context(tc.tile_pool(name="ps_h", bufs=4, space="PSUM"))
        ps_ms = ctx.enter_context(tc.tile_pool(name="ps_ms", bufs=2, space="PSUM"))
        ps_y = ctx.enter_context(tc.tile_pool(name="ps_y", bufs=2, space="PSUM"))

        # ---- persistent SBUF tiles ----
        w1t = persist.tile([128, 8 * 1024], FP8)
        w1t3 = w1t[:].rearrange("p (j e) -> p j e", j=8)
        xt = persist.tile([128, 8 * TB], FP8)
        xt3 = xt[:].rearrange("p (j t) -> p j t", j=8)
        w2t = persist.tile([128, 8 * 1024], FP8)
        w2t3 = w2t[:].rearrange("p (j c) -> p j c", j=8)
        h_sb = persist.tile([128, T * 512], BF16)
        h3 = h_sb[:].rearrange("p (t x) -> p t x", x=512)
        m_buf = persist.tile([128, T * 512], BF16)
        m3 = m_buf[:].rearrange("p (t x) -> p t x", x=512)

        w1src = w1t_d[:].rearrange("(dj p) e -> p dj e", p=128)
        xsrc = xt_d[:].rearrange("(dj p) t -> p dj t", p=128)
        w2src = w2t_d[:].rearrange("(ej p) c -> p ej c", p=128)

        # ---- prologue: parallel DMA issue on three rings, ordered by need.
        # W1's two e-halves go on different rings so the full 1MB lands in
        # about half the single-ring time; x chunks are interleaved so each
        # arrives just before the mm1 group that consumes it.  The one ACT
        # table load (natural_log_exp_and_others covers identity/copy/exp/ln)
        # is wedged between scalar-ring DMA issues it cannot delay.
        nc.scalar.dma_start(xt3[:, :, 0:256], xsrc[:, :, 0:256])
        nc.scalar.dma_start(w1t3[:, :, 512:1024], w1src[:, :, 512:1024])
        nc.scalar.add_instruction(mybir.InstLoadActFuncSet(
            name=nc.get_next_instruction_name(),
            act_func_set_id=NATURAL_LOG_EXP_SET, ins=[], outs=[]))
        nc.scalar.dma_start(xt3[:, :, 1024:1536], xsrc[:, :, 1024:1536])

        nc.sync.dma_start(w1t3[:, :, 0:512], w1src[:, :, 0:512])
        nc.sync.dma_start(xt3[:, :, 256:512], xsrc[:, :, 256:512])
        nc.sync.dma_start(xt3[:, :, 512:1024], xsrc[:, :, 512:1024])
        nc.sync.dma_start(xt3[:, :, 1536:2048], xsrc[:, :, 1536:2048])

        # gpsimd/SWDGE ring: small biases early, W2 (needed only by mm2).
        io = small.tile([128, 128], mybir.dt.int32)
        nc.gpsimd.iota(io[:], pattern=[[1, 128]], base=0, channel_multiplier=-1)
        b1_sb = small.tile([128, 8], FP32)
        nc.gpsimd.dma_start(b1_sb[:], b1_d.rearrange("(j p) -> p j", p=128))
        b2_sb = small.tile([1, C], FP32)
        nc.gpsimd.dma_start(b2_sb[:], b2_d.rearrange("(a c) -> a c", a=1))
        nc.gpsimd.dma_start(w2t3[:, :, 0:C], w2src[:, :, :])

        # vector: constants
        ident = small.tile([128, 128], BF16)
        nc.vector.tensor_scalar(ident[:], io[:], 0, None, op0=OP.is_equal)
        wstA = small.tile([128, 512], BF16)
        wstB = small.tile([128, 512], BF16)
        nc.vector.memset(wstA[:], 0.0)
        ones = small.tile([1, BC], BF16)
        nc.vector.memset(ones[:], 1.0)

        # scalar: bias prep (waits on the gpsimd b1/b2 DMAs)
        b1h = small.tile([128, 8], FP32)
        nc.scalar.mul(b1h[:], b1_sb[:], 0.5)
        b2s = small.tile([1, C], BF16)
        nc.scalar.mul(b2s[:], b2_sb[:], float(T) * W1_PRESCALE)

        # ---- matmul1: h[e, tb] = W1 @ x.T, fused 0.5*h + 0.5*b1 into scan layout ----
        def mm1_group(g, t0, tcnt):
            n = tcnt * 64
            for j in range(8):
                ps = ps_h.tile([128, 512], FP32, tag="ps_h", name=f"psh_{g}_{j}")
                for dp in range(4):   # pairs of contraction tiles (DoubleRow)
                    nc.tensor.matmul(
                        ps[:, 0:n],
                        w1t3[:, 2 * dp:2 * dp + 2, j * 128:(j + 1) * 128],
                        xt3[:, 2 * dp:2 * dp + 2, t0 * 64:(t0 + tcnt) * 64],
                        start=(dp == 0), stop=(dp == 3),
                        perf_mode=mybir.MatmulPerfMode.DoubleRow,
                    )
                nc.scalar.activation(
                    h3[:, t0:t0 + tcnt, j * 64:(j + 1) * 64],
                    ps[:, 0:n].rearrange("p (t b) -> p t b", t=tcnt),
                    AF.Identity, scale=0.5 / W1_PRESCALE, bias=b1h[:, j:j + 1],
                )

        # ---- LIF scan (DVE only, ping-pong state) ----
        # Spike-count accumulation is split at TSPLIT so the first mm2 sweep
        # (over t < TSPLIT) runs under the scan tail, keeping the PE warm and
        # shortening the serial epilogue.
        lif = _lif_op()
        TSPLIT = 24
        msumA = ps_ms.tile([128, 512], FP32, tag="ps_ms", name="msumA")
        msumB = ps_ms.tile([128, 512], FP32, tag="ps_ms", name="msumB")

        def scan_steps(t0, t1):
            for t in range(t0, t1):
                src = wstA if t % 2 == 0 else wstB
                dst = wstB if t % 2 == 0 else wstA
                nc.vector._custom_dve(lif, out=dst[:], in0=src[:],
                                      in1=h_sb[:, t * 512:(t + 1) * 512],
                                      s0=1.0, s1=0.5)
                nc.vector.tensor_scalar(m3[:, t, :], dst[:], 1.0, None, op0=OP.is_lt)

        def msum_run(t0, t1):
            for t in range(t0, t1):
                ms = msumA if t < TSPLIT else msumB
                nc.tensor.matmul(ms[:], ident[:], m3[:, t, :],
                                 start=(t in (0, TSPLIT)),
                                 stop=(t in (TSPLIT - 1, T - 1)))

        # mm2 machinery: two sweeps (ssqA over t<TSPLIT, ssqB over the rest)
        # accumulating into the same psy banks; only the B sweep sits on the
        # critical tail.  Spike counts per sweep are small ints, exact in fp8.
        k_out = 1.0 / (float(T) * W1_PRESCALE)
        psys = [(ps_y.tile([BC, 512], FP32, tag="ps_y", name=f"psy{h}"),
                 h * 512, 512 if h == 0 else C - 512) for h in range(2)]
        ej = small.tile([BC, 512], BF16)      # exp main output (value unused)
        se = small.tile([BC, 2], FP32)        # per-half row sums of exp

        def mm2_sweep(ssq3, first, last):
            for psy, c0, n in psys:
                for pj in range(4):   # DoubleRow pairs of e-tiles
                    nc.tensor.matmul(
                        psy[:, 0:n],
                        ssq3[:, 2 * pj:2 * pj + 2, :],
                        w2t3[:, 2 * pj:2 * pj + 2, c0:c0 + n],
                        start=(first and pj == 0), stop=False,
                        perf_mode=mybir.MatmulPerfMode.DoubleRow,
                    )
                if last:
                    nc.tensor.matmul(psy[:, 0:n], ones[:], b2s[:, c0:c0 + n],
                                     start=False, stop=True)
                    # |y| <= D/T + |b2| ~ 33: exp stays in fp32 range
                    half = c0 // 512
                    nc.scalar.activation(ej[:, 0:n], psy[:, 0:n], AF.Exp,
                                         scale=k_out,
                                         accum_out=se[:, half:half + 1])

        # Group sizes: tiny first group so the scan starts as early as
        # possible, a small last group so the scan tail after mm1 is short.
        mm1_group(0, 0, 2)
        mm1_group(1, 2, 6)
        scan_steps(0, 2)
        msum_run(0, 2)
        mm1_group(2, 8, 8)
        scan_steps(2, 8)
        msum_run(2, 8)
        mm1_group(3, 16, 8)
        scan_steps(8, 16)
        msum_run(8, 16)
        mm1_group(4, 24, 6)
        scan_steps(16, 24)
        msum_run(16, 24)
        mm1_group(5, 30, 2)
        ssqA = small.tile([128, 512], FP8)
        nc.scalar.activation(ssqA[:], msumA[:], AF.Copy,
                             scale=-1.0, bias=float(TSPLIT))
        mm2_sweep(ssqA[:].rearrange("p (j b) -> p j b", j=8), True, False)
        scan_steps(24, 32)
        msum_run(24, 32)
        ssqB = small.tile([128, 512], FP8)
        nc.scalar.activation(ssqB[:], msumB[:], AF.Copy,
                             scale=-1.0, bias=float(T - TSPLIT))
        mm2_sweep(ssqB[:].rearrange("p (j b) -> p j b", j=8), False, True)

        set_ = small.tile([BC, 1], FP32)
        nc.vector.tensor_tensor(set_[:], se[:, 0:1], se[:, 1:2], OP.add)
        lse = small.tile([BC, 1], FP32)
        nc.scalar.activation(lse[:], set_[:], AF.Ln)
        nlse = small.tile([BC, 1], FP32)
        nc.vector.tensor_scalar_mul(nlse[:], lse[:], -1.0)

        # out = y - lse, computed per half on two engines in parallel and
        # DMA'd out on two rings so the final write starts as early as possible
        out_sb = small.tile([BC, C], FP32)
        psy0, c0_0, n0 = psys[0]
        psy1, c0_1, n1 = psys[1]
        nc.vector.tensor_scalar(out_sb[:, c0_0:c0_0 + n0], psy0[:, 0:n0],
                                k_out, lse[:], op0=OP.mult, op1=OP.subtract)
        nc.scalar.activation(out_sb[:, c0_1:c0_1 + n1], psy1[:, 0:n1],
                             AF.Identity, scale=k_out, bias=nlse[:])
        nc.sync.dma_start(y_d[:, c0_0:c0_0 + n0], out_sb[:, c0_0:c0_0 + n0])
        nc.scalar.dma_start(y_d[:, c0_1:c0_1 + n1], out_sb[:, c0_1:c0_1 + n1])

    nc.compile()
    return nc


_CACHE = {}


def kernel(x, W1, b1, W2, b2):
    if "nc" not in _CACHE:
        _CACHE["nc"] = build_program()
    nc = _CACHE["nc"]

    f8 = mybir.dt.np(FP8)
    x = np.asarray(x, dtype=np.float32)
    w1t = np.ascontiguousarray(
        (np.asarray(W1, dtype=np.float32).T * W1_PRESCALE).astype(f8))
    w2t = np.ascontiguousarray(
        (np.asarray(W2, dtype=np.float32).T * W1_PRESCALE).astype(f8))
    b1 = np.ascontiguousarray(b1, dtype=np.float32)
    b2 = np.ascontiguousarray(b2, dtype=np.float32)
    in_maps = []
    for i in range(N_CORES):
        xs = np.ascontiguousarray(
            x[:, i * BC:(i + 1) * BC, :].reshape(TB, D).T.astype(f8))
        in_maps.append({"xT": xs, "W1T": w1t, "b1": b1, "W2T": w2t, "b2": b2})

    res = run_bass_kernel_spmd(nc, in_maps, core_ids=list(range(N_CORES)),
                               **_CACHE.get("run_kwargs", {}))
    _CACHE["last_results"] = res
    out = np.concatenate([res.results[i]["y"] for i in range(N_CORES)], axis=0)
    return out
